# revision 17
# baseline (speedup 1.0000x reference)
"""Trainium2 bass kernel for nn_CM_41162966565199 (dense_cnn, dynamic filter).

Computation (per batch sample):
  filt = Conv2d(C=64 -> 9C=576, 3x3, pad=1)(gt) + bias          # dynamic filters
  out[c,h,w] = sum_j filt[c*9+j, h, w] * patches_j(gr)[c, h, w] # 3x3 dyn. filter

Strategy: pure data parallel, one sample per NeuronCore (N=8, 8 cores).

Per core:
- Conv as shift-based matmuls in fp16 (full PE rate, half the DMA/SBUF of
  fp32r): contraction (in_channel i, tap p) tiled into 5 K=128 chunks by
  pairing taps whose flat-offset delta is +1 (or +130), realized by stacking
  two shifted copies of gt on SBUF partitions 0-63 / 64-127. Output channels
  (c, j) tiled into 5 M-tiles of two j-groups each. All matmuls K=128, M=128,
  N=512.
- Dynamic-filter stage on DVE: scalar_tensor_tensor fuses (psum + bias) * gr
  reading PSUM directly; the fp16 pairwise product sum tree runs on the
  otherwise-idle GpSimd (Pool) engine so DVE never backs up PSUM recycling.
- Spatial flattening uses a 1-ring padded 130x130 grid so every 3x3 tap is a
  pure flat offset; host pre-pads (zero ring for conv input, the replicate
  ring of gr IS the 130-grid) and crops the 130x130 output grid to 128x128.
  The upper/lower partition halves hold disjoint partial sums, folded on the
  host.
- First block has 1 N-tile so the PE starts after ~0.5 MB of DMA instead of
  ~5 MB; gr/bias DMAs issue from the ACT queue and out-DMAs from the Pool
  queue so the SP queue only carries gt windows + weights.
"""

import numpy as np

import concourse.bass as bass
import concourse.mybir as mybir
import concourse.tile as tile
from concourse import bacc
from concourse.bass_utils import run_bass_kernel_spmd
from concourse.vector_clock import ScopedClock

# ---------------------------------------------------------------- constants
N, C, H, W, KS = 8, 64, 128, 128, 3
W2 = W + 2                      # 130: 1-ring padded row width
NTILE = 512
NT = 33                         # spatial tiles; out o = h*130+w, max 16637
OUT_LEN = NT * NTILE            # 16896
FLAT_SRC = 17160                # padded flat source length (covers max reads)

F32 = mybir.dt.float32
F16 = mybir.dt.float16
ADD = mybir.AluOpType.add
MULT = mybir.AluOpType.mult

# 5 K-chunks over the 9 conv taps p=(kh,kw); flat offset d_p = kh*130+kw.
# Pairs (p_a, p_b): upper/lower SBUF partition halves. Chunks 0-2 pair
# (kh,0)+(kh,1) (delta=1, gtAB buffer), chunk 3 pairs (0,2)+(1,2)
# (delta=130, gtAC buffer), chunk 4 is the lone (2,2) with zeroed lower
# weights.
CHUNKS = [((0, 0), (0, 1)), ((1, 0), (1, 1)), ((2, 0), (2, 1)),
          ((0, 2), (1, 2)), ((2, 2), None)]
# 5 M-tiles: which two j-groups (of the 9 output filter taps) share a PSUM
# tile's upper/lower 64 partitions.
MTILES = CHUNKS

# block sizes (N-tiles per block; windows + weights reused within a block).
# Small first block -> PE starts early; small last blocks -> short tail.
SIZES = [1, 3, 4, 4, 4, 4, 4, 4, 4, 1]
assert sum(SIZES) == NT


# ------------------------------------------------- TileContext drain patch
# This walrus build rejects >2 sync-wait commands on one CTRL instruction;
# the stock TileContext tail hangs every pending sem wait on a single SP
# Drain. Split them across single-wait SP NOPs (program order on SP still
# places them before the barrier + sem reset).
def _drain_and_barrier_split(self, tick_clock, wait_clock):
    nc = self.nc
    drain_inst = nc.sync.drain()
    wait_clock.add_sem_waits(
        drain_inst.ins, ScopedClock({None: tick_clock.global_clock})
    )
    si = drain_inst.ins.sync_info
    if si is not None and len(si.on_wait) > 1:
        waits = list(si.on_wait)
        drain_inst.ins.sync_info = mybir.SyncInfo(on_wait=[waits[0]], on_update=[])
        for w in waits[1:]:
            nop = nc.sync.nop()
            nop.ins.sync_info = mybir.SyncInfo(on_wait=[w], on_update=[])
    nc.all_engine_barrier()
    assert self.sems is not None
    popped = nc._tile_sem_poison_stack.pop()
    assert popped is self._sem_poison
    # The stock tail ends with a second all_engine_barrier after the
    # semaphore clear. It only protects a following tile scope (none here)
    # and costs ~6us of measured exec time (slow staggered barrier); the
    # runtime already waits for every engine to retire, and the clear is the
    # final instruction on its engine, so re-execution stays safe.
    nc.clear_and_free_semaphores(list(self.sems.allocated().values()))


tile.TileContext._drain_and_barrier = _drain_and_barrier_split


# ------------------------------------------------------------- host prep
def _prep_gt(gt):
    """[C,H,W] -> [C, FLAT_SRC] fp16 flat 130x130 grid, 1-ring zero pad."""
    pad = np.zeros((C, W2, W2), np.float16)
    pad[:, 1:1 + H, 1:1 + W] = gt
    buf = np.zeros((C, FLAT_SRC), np.float16)
    buf[:, :W2 * W2] = pad.reshape(C, -1)
    return buf


def _prep_gr(gr):
    """[C,H,W] -> fp16 flat 130x130 grid = replicate-padded gr."""
    rp = np.pad(gr, ((0, 0), (1, 1), (1, 1)), mode="edge")
    buf = np.zeros((C, FLAT_SRC), np.float16)
    buf[:, :W2 * W2] = rp.reshape(C, -1)
    return buf


def _jidx(j):
    return j[0] * 3 + j[1]


def _prep_w(Wc):
    """[576,64,3,3] -> fp16 [128, 25*128] lhsT blocks [(m,chunk), K, M]."""
    out = np.zeros((5, 5, 128, 128), np.float32)
    cc = np.arange(C)
    for m, (j0, j1) in enumerate(MTILES):
        for c, (pa, pb) in enumerate(CHUNKS):
            for hk, p in ((0, pa), (1, pb)):
                if p is None:
                    continue
                kh, kw = p
                for hm, j in ((0, j0), (1, j1)):
                    if j is None:
                        continue
                    blk = Wc[cc * 9 + _jidx(j), :, kh, kw]  # [c_out, i]
                    out[m, c, 64 * hk:64 * hk + 64, 64 * hm:64 * hm + 64] = blk.T
    # partition-major [128, 25*128] so the device load is plain 2D DMAs
    return np.ascontiguousarray(
        out.reshape(25, 128, 128).transpose(1, 0, 2).reshape(128, 25 * 128)
    ).astype(np.float16)


def _prep_b(bc):
    """[576] -> [128,5] per-M-tile per-partition bias (partition-major)."""
    out = np.zeros((5, 128), np.float32)
    cc = np.arange(C)
    for m, (j0, j1) in enumerate(MTILES):
        for hm, j in ((0, j0), (1, j1)):
            if j is None:
                continue
            out[m, 64 * hm:64 * hm + 64] = bc[cc * 9 + _jidx(j)]
    return np.ascontiguousarray(out.T)


# --------------------------------------------------------- bass program
def _build():
    # Bacc (not plain Bass): its finalize() -> compile() legalizes the
    # multi-wait instructions Tile emits (move_matmul_waits_to_ldweights,
    # generate_event_semaphores) which this walrus build otherwise rejects
    # with "Too many sync wait commands".
    nc = bacc.Bacc(None, target_bir_lowering=False)
    gt_src = nc.dram_tensor("gt_src", [C, FLAT_SRC], F16, kind="ExternalInput")
    gr_src = nc.dram_tensor("gr_src", [C, FLAT_SRC], F16, kind="ExternalInput")
    w_src = nc.dram_tensor("w_src", [128, 25 * 128], F16, kind="ExternalInput")
    b_src = nc.dram_tensor("b_src", [128, 5], F32, kind="ExternalInput")
    o_dst = nc.dram_tensor("o_dst", [128, OUT_LEN], F16, kind="ExternalOutput")

    blocks = []
    t0 = 0
    for nb in SIZES:
        blocks.append((t0, nb))
        t0 += nb

    with tile.TileContext(nc) as tc:
        with (
            tc.tile_pool(name="wpool", bufs=1) as wpool,
            tc.tile_pool(name="winpool", bufs=2) as winpool,
            tc.tile_pool(name="pspool", bufs=4, space="PSUM") as pspool,
            tc.tile_pool(name="prodpool", bufs=12) as prodpool,
            tc.tile_pool(name="accpool", bufs=6) as accpool,
        ):
            wsb = wpool.tile([128, 25 * 128], F16, name="wsb", tag="wsb")
            bias_sb = wpool.tile([128, 5], F32, name="bias_sb", tag="bias")

            # PE warmup: the HAM clock gate keeps the PE at 1.2 GHz until it
            # has been busy ~3.4us. Issue small matmuls on a zeroed tile
            # during the head DMA wait; count sized to end right as the first
            # real matmul's inputs land (a gap would re-throttle the PE).
            warm = wpool.tile([128, 64], F16, name="warm", tag="warm")
            nc.gpsimd.memset(warm[:, :], 0.0)
            pswarm = pspool.tile([128, 2 * NTILE], F32, name="pswarm",
                                 tag="ps")
            for _ in range(30):
                nc.tensor.matmul(pswarm[0:64, 0:64], warm[:, :], warm[:, :],
                                 start=True, stop=True)

            def load_weights_m(m):
                nc.sync.dma_start(
                    out=wsb[:, m * 640:(m + 1) * 640],
                    in_=w_src[:, m * 640:(m + 1) * 640],
                )

            def stt(out_ap, ps_ap, b_ap, gr_ap):
                nc.vector.scalar_tensor_tensor(
                    out_ap, ps_ap, b_ap, gr_ap, op0=ADD, op1=MULT
                )

            def win_load(eng, pool, name, src, base, pair_step, dtype, wneed):
                """Partitions 0-63 <- src[base+q], 64-127 <-
                src[base+pair_step+q], as two 2D DMAs of just the columns
                this block touches. eng picks the issuing queue."""
                t = pool.tile([128, 4 * NTILE + 262], dtype, name=name, tag=name)
                eng.dma_start(out=t[0:64, 0:wneed],
                              in_=src[:, base:base + wneed])
                eng.dma_start(
                    out=t[64:128, 0:wneed],
                    in_=src[:, base + pair_step:base + pair_step + wneed],
                )
                return t

            for bi, (t0, nb) in enumerate(blocks):
                T = t0 * NTILE
                wab = nb * NTILE + 262
                wac = nb * NTILE
                # gt windows + weights on SP queue; gr windows + bias on the
                # ACT queue so the first-matmul deps don't queue behind them.
                if bi == 0:
                    # first M-tile's weights lead the SP queue (smallest
                    # first-matmul dep); bias leads the ACT queue
                    load_weights_m(0)
                    nc.scalar.dma_start(out=bias_sb[:, :], in_=b_src[:, :])
                gtab = win_load(nc.sync, winpool, "gtab", gt_src, T, 1, F16, wab)
                gtac = win_load(nc.sync, winpool, "gtac", gt_src, T + 2, 130,
                                F16, wac)
                grab = win_load(nc.scalar, winpool, "grab", gr_src, T, 1, F16,
                                wab)
                grac = win_load(nc.scalar, winpool, "grac", gr_src, T + 2, 130,
                                F16, wac)
                if bi == 0:
                    for m in range(1, 5):
                        load_weights_m(m)

                # Conv matmuls per M-tile (weights reused across the block's
                # N-tiles; the PE reorder window pulls LDWEIGHTS under the
                # previous matmul). PSUM tiles span TWO banks (two adjacent
                # N-tiles) so the product stage runs 1024-wide DVE ops.
                # Products are written fp16 so the Pool add tree runs 2x.
                # fp16 sum tree engine: Pool (slow but fully overlapped) for
                # mid blocks; DVE for the final block so the kernel tail is
                # short (there is no next block for Pool's latency to stall).
                adde = nc.vector if bi >= len(blocks) - 2 else nc.gpsimd

                npair = (nb + 1) // 2
                prods = [[None] * 5 for _ in range(npair)]
                accs = [[None, None] for _ in range(npair)]
                for m in range(5):
                    pst = [
                        pspool.tile([128, 2 * NTILE], F32, name=f"ps{m}_{p}",
                                    tag="ps")
                        for p in range(npair)
                    ]
                    for c in range(5):
                        k = m * 5 + c
                        lhsT = wsb[:, k * 128:(k + 1) * 128]
                        for tb in range(nb):
                            q = tb * NTILE
                            if c < 3:
                                rhs = gtab[:, q + c * W2: q + c * W2 + NTILE]
                            elif c == 3:
                                rhs = gtac[:, q: q + NTILE]
                            else:
                                rhs = gtab[:, q + 262: q + 262 + NTILE]
                            out_ps = pst[tb // 2][:, (tb % 2) * NTILE:
                                                  (tb % 2 + 1) * NTILE]
                            nc.tensor.matmul(
                                out_ps, lhsT, rhs,
                                start=(c == 0), stop=(c == 4),
                            )
                    for p in range(npair):
                        q = 2 * p * NTILE
                        Wd = min(2 * NTILE, (nb - 2 * p) * NTILE)
                        pr = prodpool.tile(
                            [128, 2 * NTILE], F16, name=f"m{m}", tag="prod"
                        )
                        prods[p][m] = pr
                        if m < 3:
                            stt(pr[:, 0:Wd], pst[p][:, 0:Wd],
                                bias_sb[:, m:m + 1],
                                grab[:, q + m * W2: q + m * W2 + Wd])
                        elif m == 3:
                            stt(pr[:, 0:Wd], pst[p][:, 0:Wd],
                                bias_sb[:, 3:4],
                                grac[:, q: q + Wd])
                        else:
                            stt(pr[0:64, 0:Wd], pst[p][0:64, 0:Wd],
                                bias_sb[0:64, 4:5],
                                grab[0:64, q + 262: q + 262 + Wd])
                    # eager leaf adds: a1 as soon as m0/m1 products exist,
                    # a2 after m2/m3 — keeps the add engine streaming instead
                    # of bursting at block end
                    if m in (1, 3):
                        for p in range(npair):
                            Wd = min(2 * NTILE, (nb - 2 * p) * NTILE)
                            a = accpool.tile([128, 2 * NTILE], F16,
                                             name=f"a{m}", tag="acc")
                            accs[p][m // 2] = a
                            adde.tensor_tensor(a[:, 0:Wd],
                                               prods[p][m - 1][:, 0:Wd],
                                               prods[p][m][:, 0:Wd], op=ADD)

                for p in range(npair):
                    t = t0 + 2 * p
                    Wd = min(2 * NTILE, (nb - 2 * p) * NTILE)
                    a1, a2 = accs[p]
                    a3 = accpool.tile([128, 2 * NTILE], F16, name="a3",
                                      tag="acc")
                    adde.tensor_tensor(a3[:, 0:Wd], a1[:, 0:Wd],
                                       a2[:, 0:Wd], op=ADD)
                    adde.tensor_tensor(a3[0:64, 0:Wd], a3[0:64, 0:Wd],
                                       prods[p][4][0:64, 0:Wd], op=ADD)
                    nc.scalar.dma_start(
                        out=o_dst[:, t * NTILE: t * NTILE + Wd],
                        in_=a3[:, 0:Wd],
                    )
    nc.finalize()
    return nc


_NC = None


def _get_nc():
    global _NC
    if _NC is None:
        _NC = _build()
    return _NC


_RUN_KW = {}  # test harness can inject trace=True etc.
_LAST_RESULT = None


def kernel(gr, gt, Wc, bc):
    global _LAST_RESULT
    gr = np.ascontiguousarray(np.asarray(gr, dtype=np.float32))
    gt = np.ascontiguousarray(np.asarray(gt, dtype=np.float32))
    Wc = np.asarray(Wc, dtype=np.float32)
    bc = np.asarray(bc, dtype=np.float32)

    wb = _prep_w(Wc)
    bb = _prep_b(bc)
    in_maps = [
        {
            "gt_src": _prep_gt(gt[n]),
            "gr_src": _prep_gr(gr[n]),
            "w_src": wb,
            "b_src": bb,
        }
        for n in range(N)
    ]
    res = run_bass_kernel_spmd(
        _get_nc(), in_maps, core_ids=list(range(N)), **_RUN_KW
    )
    _LAST_RESULT = res

    hh = np.arange(H)
    cols = (hh * W2)[:, None] + np.arange(W)[None, :]
    outs = []
    for n in range(N):
        O = res.results[n]["o_dst"].astype(np.float32)
        flat = O[:64] + O[64:]
        outs.append(flat[:, cols])
    return np.stack(outs).astype(np.float32)


# revision 19
# speedup vs baseline: 1.0120x; 1.0120x over previous
"""Trainium2 bass kernel for nn_CM_41162966565199 (dense_cnn, dynamic filter).

Computation (per batch sample):
  filt = Conv2d(C=64 -> 9C=576, 3x3, pad=1)(gt) + bias          # dynamic filters
  out[c,h,w] = sum_j filt[c*9+j, h, w] * patches_j(gr)[c, h, w] # 3x3 dyn. filter

Strategy: pure data parallel, one sample per NeuronCore (N=8, 8 cores).

Per core:
- Conv as shift-based matmuls in fp16 (full PE rate, half the DMA/SBUF of
  fp32r): contraction (in_channel i, tap p) tiled into 5 K=128 chunks by
  pairing taps whose flat-offset delta is +1 (or +130), realized by stacking
  two shifted copies of gt on SBUF partitions 0-63 / 64-127. Output channels
  (c, j) tiled into 5 M-tiles of two j-groups each. All matmuls K=128, M=128,
  N=512.
- Dynamic-filter stage on DVE: scalar_tensor_tensor fuses (psum + bias) * gr
  reading PSUM directly; the fp16 pairwise product sum tree runs on the
  otherwise-idle GpSimd (Pool) engine so DVE never backs up PSUM recycling.
- Spatial flattening uses a 1-ring padded 130x130 grid so every 3x3 tap is a
  pure flat offset; host pre-pads (zero ring for conv input, the replicate
  ring of gr IS the 130-grid) and crops the 130x130 output grid to 128x128.
  The upper/lower partition halves hold disjoint partial sums, folded on the
  host.
- First block has 1 N-tile so the PE starts after ~0.5 MB of DMA instead of
  ~5 MB; gr/bias DMAs issue from the ACT queue and out-DMAs from the Pool
  queue so the SP queue only carries gt windows + weights.
"""

import numpy as np

import concourse.bass as bass
import concourse.mybir as mybir
import concourse.tile as tile
from concourse import bacc
from concourse.bass_utils import run_bass_kernel_spmd
from concourse.vector_clock import ScopedClock

# ---------------------------------------------------------------- constants
N, C, H, W, KS = 8, 64, 128, 128, 3
W2 = W + 2                      # 130: 1-ring padded row width
NTILE = 512
NT = 33                         # spatial tiles; out o = h*130+w, max 16637
OUT_LEN = NT * NTILE            # 16896
FLAT_SRC = 17160                # padded flat source length (covers max reads)

F32 = mybir.dt.float32
F16 = mybir.dt.float16
ADD = mybir.AluOpType.add
MULT = mybir.AluOpType.mult

# 5 K-chunks over the 9 conv taps p=(kh,kw); flat offset d_p = kh*130+kw.
# Pairs (p_a, p_b): upper/lower SBUF partition halves. Chunks 0-2 pair
# (kh,0)+(kh,1) (delta=1, gtAB buffer), chunk 3 pairs (0,2)+(1,2)
# (delta=130, gtAC buffer), chunk 4 is the lone (2,2) with zeroed lower
# weights.
CHUNKS = [((0, 0), (0, 1)), ((1, 0), (1, 1)), ((2, 0), (2, 1)),
          ((0, 2), (1, 2)), ((2, 2), None)]
# 5 M-tiles: which two j-groups (of the 9 output filter taps) share a PSUM
# tile's upper/lower 64 partitions.
MTILES = CHUNKS

# block sizes (N-tiles per block; windows + weights reused within a block).
# Small first block -> PE starts early; small last blocks -> short tail.
SIZES = [1, 3, 4, 4, 4, 4, 4, 4, 4, 1]
assert sum(SIZES) == NT


# ------------------------------------------------- TileContext drain patch
# This walrus build rejects >2 sync-wait commands on one CTRL instruction;
# the stock TileContext tail hangs every pending sem wait on a single SP
# Drain. Split them across single-wait SP NOPs (program order on SP still
# places them before the barrier + sem reset).
def _drain_and_barrier_split(self, tick_clock, wait_clock):
    nc = self.nc
    drain_inst = nc.sync.drain()
    wait_clock.add_sem_waits(
        drain_inst.ins, ScopedClock({None: tick_clock.global_clock})
    )
    si = drain_inst.ins.sync_info
    if si is not None and len(si.on_wait) > 1:
        waits = list(si.on_wait)
        drain_inst.ins.sync_info = mybir.SyncInfo(on_wait=[waits[0]], on_update=[])
        for w in waits[1:]:
            nop = nc.sync.nop()
            nop.ins.sync_info = mybir.SyncInfo(on_wait=[w], on_update=[])
    nc.all_engine_barrier()
    assert self.sems is not None
    popped = nc._tile_sem_poison_stack.pop()
    assert popped is self._sem_poison
    # The stock tail ends with a second all_engine_barrier after the
    # semaphore clear. It only protects a following tile scope (none here)
    # and costs ~6us of measured exec time (slow staggered barrier); the
    # runtime already waits for every engine to retire, and the clear is the
    # final instruction on its engine, so re-execution stays safe.
    nc.clear_and_free_semaphores(list(self.sems.allocated().values()))


tile.TileContext._drain_and_barrier = _drain_and_barrier_split


# ------------------------------------------------------------- host prep
def _prep_gt(gt):
    """[C,H,W] -> [C, FLAT_SRC] fp16 flat 130x130 grid, 1-ring zero pad."""
    pad = np.zeros((C, W2, W2), np.float16)
    pad[:, 1:1 + H, 1:1 + W] = gt
    buf = np.zeros((C, FLAT_SRC), np.float16)
    buf[:, :W2 * W2] = pad.reshape(C, -1)
    return buf


def _prep_gr(gr):
    """[C,H,W] -> fp16 flat 130x130 grid = replicate-padded gr."""
    rp = np.pad(gr, ((0, 0), (1, 1), (1, 1)), mode="edge")
    buf = np.zeros((C, FLAT_SRC), np.float16)
    buf[:, :W2 * W2] = rp.reshape(C, -1)
    return buf


def _jidx(j):
    return j[0] * 3 + j[1]


def _prep_w(Wc):
    """[576,64,3,3] -> fp16 [128, 25*128] lhsT blocks [(m,chunk), K, M]."""
    out = np.zeros((5, 5, 128, 128), np.float32)
    cc = np.arange(C)
    for m, (j0, j1) in enumerate(MTILES):
        for c, (pa, pb) in enumerate(CHUNKS):
            for hk, p in ((0, pa), (1, pb)):
                if p is None:
                    continue
                kh, kw = p
                for hm, j in ((0, j0), (1, j1)):
                    if j is None:
                        continue
                    blk = Wc[cc * 9 + _jidx(j), :, kh, kw]  # [c_out, i]
                    out[m, c, 64 * hk:64 * hk + 64, 64 * hm:64 * hm + 64] = blk.T
    # partition-major [128, 25*128] so the device load is plain 2D DMAs
    return np.ascontiguousarray(
        out.reshape(25, 128, 128).transpose(1, 0, 2).reshape(128, 25 * 128)
    ).astype(np.float16)


def _prep_b(bc):
    """[576] -> [128,5] per-M-tile per-partition bias (partition-major)."""
    out = np.zeros((5, 128), np.float32)
    cc = np.arange(C)
    for m, (j0, j1) in enumerate(MTILES):
        for hm, j in ((0, j0), (1, j1)):
            if j is None:
                continue
            out[m, 64 * hm:64 * hm + 64] = bc[cc * 9 + _jidx(j)]
    return np.ascontiguousarray(out.T)


# --------------------------------------------------------- bass program
def _build():
    # Bacc (not plain Bass): its finalize() -> compile() legalizes the
    # multi-wait instructions Tile emits (move_matmul_waits_to_ldweights,
    # generate_event_semaphores) which this walrus build otherwise rejects
    # with "Too many sync wait commands".
    nc = bacc.Bacc(None, target_bir_lowering=False)
    gt_src = nc.dram_tensor("gt_src", [C, FLAT_SRC], F16, kind="ExternalInput")
    gr_src = nc.dram_tensor("gr_src", [C, FLAT_SRC], F16, kind="ExternalInput")
    w_src = nc.dram_tensor("w_src", [128, 25 * 128], F16, kind="ExternalInput")
    b_src = nc.dram_tensor("b_src", [128, 5], F32, kind="ExternalInput")
    o_dst = nc.dram_tensor("o_dst", [128, OUT_LEN], F16, kind="ExternalOutput")

    blocks = []
    t0 = 0
    for nb in SIZES:
        blocks.append((t0, nb))
        t0 += nb

    with tile.TileContext(nc) as tc:
        with (
            tc.tile_pool(name="wpool", bufs=1) as wpool,
            tc.tile_pool(name="winpool", bufs=2) as winpool,
            tc.tile_pool(name="pspool", bufs=4, space="PSUM") as pspool,
            tc.tile_pool(name="prodpool", bufs=12) as prodpool,
            tc.tile_pool(name="accpool", bufs=6) as accpool,
        ):
            wsb = wpool.tile([128, 25 * 128], F16, name="wsb", tag="wsb")
            bias_sb = wpool.tile([128, 5], F32, name="bias_sb", tag="bias")

            # PE warmup: the HAM clock gate keeps the PE at 1.2 GHz until it
            # has been busy ~3.4us. Issue small matmuls on a zeroed tile
            # during the head DMA wait; count sized to end right as the first
            # real matmul's inputs land (a gap would re-throttle the PE).
            warm = wpool.tile([128, 64], F16, name="warm", tag="warm")
            nc.gpsimd.memset(warm[:, :], 0.0)
            pswarm = pspool.tile([128, 2 * NTILE], F32, name="pswarm",
                                 tag="ps")
            for _ in range(70):
                nc.tensor.matmul(pswarm[0:64, 0:64], warm[:, :], warm[:, :],
                                 start=True, stop=True)

            def load_weights_m(m):
                nc.sync.dma_start(
                    out=wsb[:, m * 640:(m + 1) * 640],
                    in_=w_src[:, m * 640:(m + 1) * 640],
                )

            def stt(out_ap, ps_ap, b_ap, gr_ap):
                nc.vector.scalar_tensor_tensor(
                    out_ap, ps_ap, b_ap, gr_ap, op0=ADD, op1=MULT
                )

            def win_load(eng, pool, name, src, base, pair_step, dtype, wneed):
                """Partitions 0-63 <- src[base+q], 64-127 <-
                src[base+pair_step+q], as two 2D DMAs of just the columns
                this block touches. eng picks the issuing queue."""
                t = pool.tile([128, 4 * NTILE + 262], dtype, name=name, tag=name)
                eng.dma_start(out=t[0:64, 0:wneed],
                              in_=src[:, base:base + wneed])
                eng.dma_start(
                    out=t[64:128, 0:wneed],
                    in_=src[:, base + pair_step:base + pair_step + wneed],
                )
                return t

            for bi, (t0, nb) in enumerate(blocks):
                T = t0 * NTILE
                wab = nb * NTILE + 262
                wac = nb * NTILE
                # gt windows + weights on SP queue; gr windows + bias on the
                # ACT queue so the first-matmul deps don't queue behind them.
                if bi == 0:
                    # DMA completion is row-descriptor-bound (~3.4us for a
                    # 64-row window regardless of width), so the head is
                    # ordered for earliest first-matmul: w0 + bias lead the
                    # ACT queue while the gt windows lead the SP queue.
                    nc.scalar.dma_start(out=wsb[:, 0:640], in_=w_src[:, 0:640])
                    nc.scalar.dma_start(out=bias_sb[:, :], in_=b_src[:, :])
                gtab = win_load(nc.sync, winpool, "gtab", gt_src, T, 1, F16, wab)
                gtac = win_load(nc.sync, winpool, "gtac", gt_src, T + 2, 130,
                                F16, wac)
                grab = win_load(nc.scalar, winpool, "grab", gr_src, T, 1, F16,
                                wab)
                grac = win_load(nc.scalar, winpool, "grac", gr_src, T + 2, 130,
                                F16, wac)
                if bi == 0:
                    # remaining weights as one DMA so block1's windows issue
                    # soon after
                    nc.sync.dma_start(out=wsb[:, 640:3200],
                                      in_=w_src[:, 640:3200])

                # Conv matmuls per M-tile (weights reused across the block's
                # N-tiles; the PE reorder window pulls LDWEIGHTS under the
                # previous matmul). PSUM tiles span TWO banks (two adjacent
                # N-tiles) so the product stage runs 1024-wide DVE ops.
                # Products are written fp16 so the Pool add tree runs 2x.
                # fp16 sum tree engine: Pool (slow but fully overlapped) for
                # mid blocks; DVE for the final block so the kernel tail is
                # short (there is no next block for Pool's latency to stall).
                adde = nc.vector if bi >= len(blocks) - 2 else nc.gpsimd

                npair = (nb + 1) // 2
                prods = [[None] * 5 for _ in range(npair)]
                accs = [[None, None] for _ in range(npair)]
                for m in range(5):
                    pst = [
                        pspool.tile([128, 2 * NTILE], F32, name=f"ps{m}_{p}",
                                    tag="ps")
                        for p in range(npair)
                    ]
                    for c in range(5):
                        k = m * 5 + c
                        lhsT = wsb[:, k * 128:(k + 1) * 128]
                        for tb in range(nb):
                            q = tb * NTILE
                            if c < 3:
                                rhs = gtab[:, q + c * W2: q + c * W2 + NTILE]
                            elif c == 3:
                                rhs = gtac[:, q: q + NTILE]
                            else:
                                rhs = gtab[:, q + 262: q + 262 + NTILE]
                            out_ps = pst[tb // 2][:, (tb % 2) * NTILE:
                                                  (tb % 2 + 1) * NTILE]
                            nc.tensor.matmul(
                                out_ps, lhsT, rhs,
                                start=(c == 0), stop=(c == 4),
                            )
                    for p in range(npair):
                        q = 2 * p * NTILE
                        Wd = min(2 * NTILE, (nb - 2 * p) * NTILE)
                        pr = prodpool.tile(
                            [128, 2 * NTILE], F16, name=f"m{m}", tag="prod"
                        )
                        prods[p][m] = pr
                        if m < 3:
                            stt(pr[:, 0:Wd], pst[p][:, 0:Wd],
                                bias_sb[:, m:m + 1],
                                grab[:, q + m * W2: q + m * W2 + Wd])
                        elif m == 3:
                            stt(pr[:, 0:Wd], pst[p][:, 0:Wd],
                                bias_sb[:, 3:4],
                                grac[:, q: q + Wd])
                        else:
                            stt(pr[0:64, 0:Wd], pst[p][0:64, 0:Wd],
                                bias_sb[0:64, 4:5],
                                grab[0:64, q + 262: q + 262 + Wd])
                    # eager leaf adds: a1 as soon as m0/m1 products exist,
                    # a2 after m2/m3 — keeps the add engine streaming instead
                    # of bursting at block end
                    if m in (1, 3):
                        for p in range(npair):
                            Wd = min(2 * NTILE, (nb - 2 * p) * NTILE)
                            a = accpool.tile([128, 2 * NTILE], F16,
                                             name=f"a{m}", tag="acc")
                            accs[p][m // 2] = a
                            adde.tensor_tensor(a[:, 0:Wd],
                                               prods[p][m - 1][:, 0:Wd],
                                               prods[p][m][:, 0:Wd], op=ADD)

                for p in range(npair):
                    t = t0 + 2 * p
                    Wd = min(2 * NTILE, (nb - 2 * p) * NTILE)
                    a1, a2 = accs[p]
                    a3 = accpool.tile([128, 2 * NTILE], F16, name="a3",
                                      tag="acc")
                    adde.tensor_tensor(a3[:, 0:Wd], a1[:, 0:Wd],
                                       a2[:, 0:Wd], op=ADD)
                    adde.tensor_tensor(a3[0:64, 0:Wd], a3[0:64, 0:Wd],
                                       prods[p][4][0:64, 0:Wd], op=ADD)
                    nc.scalar.dma_start(
                        out=o_dst[:, t * NTILE: t * NTILE + Wd],
                        in_=a3[:, 0:Wd],
                    )
    nc.finalize()
    return nc


_NC = None


def _get_nc():
    global _NC
    if _NC is None:
        _NC = _build()
    return _NC


_RUN_KW = {}  # test harness can inject trace=True etc.
_LAST_RESULT = None


def kernel(gr, gt, Wc, bc):
    global _LAST_RESULT
    gr = np.ascontiguousarray(np.asarray(gr, dtype=np.float32))
    gt = np.ascontiguousarray(np.asarray(gt, dtype=np.float32))
    Wc = np.asarray(Wc, dtype=np.float32)
    bc = np.asarray(bc, dtype=np.float32)

    wb = _prep_w(Wc)
    bb = _prep_b(bc)
    in_maps = [
        {
            "gt_src": _prep_gt(gt[n]),
            "gr_src": _prep_gr(gr[n]),
            "w_src": wb,
            "b_src": bb,
        }
        for n in range(N)
    ]
    res = run_bass_kernel_spmd(
        _get_nc(), in_maps, core_ids=list(range(N)), **_RUN_KW
    )
    _LAST_RESULT = res

    hh = np.arange(H)
    cols = (hh * W2)[:, None] + np.arange(W)[None, :]
    outs = []
    for n in range(N):
        O = res.results[n]["o_dst"].astype(np.float32)
        flat = O[:64] + O[64:]
        outs.append(flat[:, cols])
    return np.stack(outs).astype(np.float32)


# revision 22
# speedup vs baseline: 1.0127x; 1.0007x over previous
"""Trainium2 bass kernel for nn_CM_41162966565199 (dense_cnn, dynamic filter).

Computation (per batch sample):
  filt = Conv2d(C=64 -> 9C=576, 3x3, pad=1)(gt) + bias          # dynamic filters
  out[c,h,w] = sum_j filt[c*9+j, h, w] * patches_j(gr)[c, h, w] # 3x3 dyn. filter

Strategy: pure data parallel, one sample per NeuronCore (N=8, 8 cores).

Per core:
- Conv as shift-based matmuls in fp16 (full PE rate, half the DMA/SBUF of
  fp32r): contraction (in_channel i, tap p) tiled into 5 K=128 chunks by
  pairing taps whose flat-offset delta is +1 (or +130), realized by stacking
  two shifted copies of gt on SBUF partitions 0-63 / 64-127. Output channels
  (c, j) tiled into 5 M-tiles of two j-groups each. All matmuls K=128, M=128,
  N=512.
- Dynamic-filter stage on DVE: scalar_tensor_tensor fuses (psum + bias) * gr
  reading PSUM directly; the fp16 pairwise product sum tree runs on the
  otherwise-idle GpSimd (Pool) engine so DVE never backs up PSUM recycling.
- Spatial flattening uses a 1-ring padded 130x130 grid so every 3x3 tap is a
  pure flat offset; host pre-pads (zero ring for conv input, the replicate
  ring of gr IS the 130-grid) and crops the 130x130 output grid to 128x128.
  The upper/lower partition halves hold disjoint partial sums, folded on the
  host.
- First block has 1 N-tile so the PE starts after ~0.5 MB of DMA instead of
  ~5 MB; gr/bias DMAs issue from the ACT queue and out-DMAs from the Pool
  queue so the SP queue only carries gt windows + weights.
"""

import numpy as np

import concourse.bass as bass
import concourse.mybir as mybir
import concourse.tile as tile
from concourse import bacc
from concourse.bass_utils import run_bass_kernel_spmd
from concourse.vector_clock import ScopedClock

# ---------------------------------------------------------------- constants
N, C, H, W, KS = 8, 64, 128, 128, 3
W2 = W + 2                      # 130: 1-ring padded row width
NTILE = 512
NT = 33                         # spatial tiles; out o = h*130+w, max 16637
OUT_LEN = NT * NTILE            # 16896
FLAT_SRC = 17160                # padded flat source length (covers max reads)

F32 = mybir.dt.float32
F16 = mybir.dt.float16
ADD = mybir.AluOpType.add
MULT = mybir.AluOpType.mult

# 5 K-chunks over the 9 conv taps p=(kh,kw); flat offset d_p = kh*130+kw.
# Pairs (p_a, p_b): upper/lower SBUF partition halves. Chunks 0-2 pair
# (kh,0)+(kh,1) (delta=1, gtAB buffer), chunk 3 pairs (0,2)+(1,2)
# (delta=130, gtAC buffer), chunk 4 is the lone (2,2) with zeroed lower
# weights.
CHUNKS = [((0, 0), (0, 1)), ((1, 0), (1, 1)), ((2, 0), (2, 1)),
          ((0, 2), (1, 2)), ((2, 2), None)]
# 5 M-tiles: which two j-groups (of the 9 output filter taps) share a PSUM
# tile's upper/lower 64 partitions.
MTILES = CHUNKS

# block sizes (N-tiles per block; windows + weights reused within a block).
# Small first block -> PE starts early; small last blocks -> short tail.
SIZES = [1, 3, 4, 4, 4, 4, 4, 4, 4, 1]
assert sum(SIZES) == NT


# ------------------------------------------------- TileContext drain patch
# This walrus build rejects >2 sync-wait commands on one CTRL instruction;
# the stock TileContext tail hangs every pending sem wait on a single SP
# Drain. Split them across single-wait SP NOPs (program order on SP still
# places them before the barrier + sem reset).
def _drain_and_barrier_split(self, tick_clock, wait_clock):
    nc = self.nc
    drain_inst = nc.sync.drain()
    wait_clock.add_sem_waits(
        drain_inst.ins, ScopedClock({None: tick_clock.global_clock})
    )
    si = drain_inst.ins.sync_info
    if si is not None and len(si.on_wait) > 1:
        waits = list(si.on_wait)
        drain_inst.ins.sync_info = mybir.SyncInfo(on_wait=[waits[0]], on_update=[])
        for w in waits[1:]:
            nop = nc.sync.nop()
            nop.ins.sync_info = mybir.SyncInfo(on_wait=[w], on_update=[])
    nc.all_engine_barrier()
    assert self.sems is not None
    popped = nc._tile_sem_poison_stack.pop()
    assert popped is self._sem_poison
    # The stock tail ends with a second all_engine_barrier after the
    # semaphore clear. It only protects a following tile scope (none here)
    # and costs ~6us of measured exec time (slow staggered barrier); the
    # runtime already waits for every engine to retire, and the clear is the
    # final instruction on its engine, so re-execution stays safe.
    nc.clear_and_free_semaphores(list(self.sems.allocated().values()))


tile.TileContext._drain_and_barrier = _drain_and_barrier_split


# ------------------------------------------------------------- host prep
def _prep_gt(gt):
    """[C,H,W] -> [C, FLAT_SRC] fp16 flat 130x130 grid, 1-ring zero pad."""
    pad = np.zeros((C, W2, W2), np.float16)
    pad[:, 1:1 + H, 1:1 + W] = gt
    buf = np.zeros((C, FLAT_SRC), np.float16)
    buf[:, :W2 * W2] = pad.reshape(C, -1)
    return buf


def _prep_gr(gr):
    """[C,H,W] -> fp16 flat 130x130 grid = replicate-padded gr."""
    rp = np.pad(gr, ((0, 0), (1, 1), (1, 1)), mode="edge")
    buf = np.zeros((C, FLAT_SRC), np.float16)
    buf[:, :W2 * W2] = rp.reshape(C, -1)
    return buf


def _jidx(j):
    return j[0] * 3 + j[1]


def _prep_w(Wc):
    """[576,64,3,3] -> fp16 [128, 25*128] lhsT blocks [(m,chunk), K, M]."""
    out = np.zeros((5, 5, 128, 128), np.float32)
    cc = np.arange(C)
    for m, (j0, j1) in enumerate(MTILES):
        for c, (pa, pb) in enumerate(CHUNKS):
            for hk, p in ((0, pa), (1, pb)):
                if p is None:
                    continue
                kh, kw = p
                for hm, j in ((0, j0), (1, j1)):
                    if j is None:
                        continue
                    blk = Wc[cc * 9 + _jidx(j), :, kh, kw]  # [c_out, i]
                    out[m, c, 64 * hk:64 * hk + 64, 64 * hm:64 * hm + 64] = blk.T
    # partition-major [128, 25*128] so the device load is plain 2D DMAs
    return np.ascontiguousarray(
        out.reshape(25, 128, 128).transpose(1, 0, 2).reshape(128, 25 * 128)
    ).astype(np.float16)


def _prep_b(bc):
    """[576] -> [128,5] per-M-tile per-partition bias (partition-major)."""
    out = np.zeros((5, 128), np.float32)
    cc = np.arange(C)
    for m, (j0, j1) in enumerate(MTILES):
        for hm, j in ((0, j0), (1, j1)):
            if j is None:
                continue
            out[m, 64 * hm:64 * hm + 64] = bc[cc * 9 + _jidx(j)]
    return np.ascontiguousarray(out.T)


# --------------------------------------------------------- bass program
def _build():
    # Bacc (not plain Bass): its finalize() -> compile() legalizes the
    # multi-wait instructions Tile emits (move_matmul_waits_to_ldweights,
    # generate_event_semaphores) which this walrus build otherwise rejects
    # with "Too many sync wait commands".
    nc = bacc.Bacc(None, target_bir_lowering=False)
    gt_src = nc.dram_tensor("gt_src", [C, FLAT_SRC], F16, kind="ExternalInput")
    gr_src = nc.dram_tensor("gr_src", [C, FLAT_SRC], F16, kind="ExternalInput")
    w_src = nc.dram_tensor("w_src", [128, 25 * 128], F16, kind="ExternalInput")
    b_src = nc.dram_tensor("b_src", [128, 5], F32, kind="ExternalInput")
    o_dst = nc.dram_tensor("o_dst", [128, OUT_LEN], F16, kind="ExternalOutput")

    blocks = []
    t0 = 0
    for nb in SIZES:
        blocks.append((t0, nb))
        t0 += nb

    with tile.TileContext(nc) as tc:
        with (
            tc.tile_pool(name="wpool", bufs=1) as wpool,
            tc.tile_pool(name="winpool", bufs=2) as winpool,
            tc.tile_pool(name="pspool", bufs=4, space="PSUM") as pspool,
            tc.tile_pool(name="prodpool", bufs=12) as prodpool,
            tc.tile_pool(name="accpool", bufs=6) as accpool,
        ):
            wsb = wpool.tile([128, 25 * 128], F16, name="wsb", tag="wsb")
            bias_sb = wpool.tile([128, 5], F32, name="bias_sb", tag="bias")

            # PE warmup: the HAM clock gate keeps the PE at 1.2 GHz until it
            # has been busy ~3.4us. Issue small matmuls on a zeroed tile
            # during the head DMA wait; count sized to end right as the first
            # real matmul's inputs land (a gap would re-throttle the PE).
            warm = wpool.tile([128, 64], F16, name="warm", tag="warm")
            nc.gpsimd.memset(warm[:, :], 0.0)
            pswarm = pspool.tile([128, 2 * NTILE], F32, name="pswarm",
                                 tag="ps")
            for _ in range(100):
                nc.tensor.matmul(pswarm[0:64, 0:64], warm[:, :], warm[:, :],
                                 start=True, stop=True)

            def load_weights_m(m):
                nc.sync.dma_start(
                    out=wsb[:, m * 640:(m + 1) * 640],
                    in_=w_src[:, m * 640:(m + 1) * 640],
                )

            def stt(out_ap, ps_ap, b_ap, gr_ap):
                nc.vector.scalar_tensor_tensor(
                    out_ap, ps_ap, b_ap, gr_ap, op0=ADD, op1=MULT
                )

            def win_load(eng, pool, name, src, base, pair_step, dtype, wneed):
                """Partitions 0-63 <- src[base+q], 64-127 <-
                src[base+pair_step+q], as two 2D DMAs of just the columns
                this block touches. eng picks the issuing queue."""
                t = pool.tile([128, 4 * NTILE + 262], dtype, name=name, tag=name)
                eng.dma_start(out=t[0:64, 0:wneed],
                              in_=src[:, base:base + wneed])
                eng.dma_start(
                    out=t[64:128, 0:wneed],
                    in_=src[:, base + pair_step:base + pair_step + wneed],
                )
                return t

            for bi, (t0, nb) in enumerate(blocks):
                T = t0 * NTILE
                wab = nb * NTILE + 262
                wac = nb * NTILE
                # gt windows + weights on SP queue; gr windows + bias on the
                # ACT queue so the first-matmul deps don't queue behind them.
                if bi == 0:
                    # DMA completion is row-descriptor-bound (~3.4us for a
                    # 64-row window regardless of width), so the head is
                    # ordered for earliest first-matmul: w0 + bias lead the
                    # ACT queue while the gt windows lead the SP queue.
                    nc.scalar.dma_start(out=wsb[:, 0:640], in_=w_src[:, 0:640])
                    nc.scalar.dma_start(out=bias_sb[:, :], in_=b_src[:, :])
                gtab = win_load(nc.sync, winpool, "gtab", gt_src, T, 1, F16, wab)
                gtac = win_load(nc.sync, winpool, "gtac", gt_src, T + 2, 130,
                                F16, wac)
                grab = win_load(nc.scalar, winpool, "grab", gr_src, T, 1, F16,
                                wab)
                grac = win_load(nc.scalar, winpool, "grac", gr_src, T + 2, 130,
                                F16, wac)
                if bi == 0:
                    # remaining weights split to match the m-loop's
                    # consumption ladder (DMA completion is latency-bound)
                    nc.sync.dma_start(out=wsb[:, 640:1280],
                                      in_=w_src[:, 640:1280])
                    nc.sync.dma_start(out=wsb[:, 1280:1920],
                                      in_=w_src[:, 1280:1920])
                    nc.sync.dma_start(out=wsb[:, 1920:3200],
                                      in_=w_src[:, 1920:3200])

                # Conv matmuls per M-tile (weights reused across the block's
                # N-tiles; the PE reorder window pulls LDWEIGHTS under the
                # previous matmul). PSUM tiles span TWO banks (two adjacent
                # N-tiles) so the product stage runs 1024-wide DVE ops.
                # Products are written fp16 so the Pool add tree runs 2x.
                # fp16 sum tree engine: Pool (slow but fully overlapped) for
                # mid blocks; DVE for the final block so the kernel tail is
                # short (there is no next block for Pool's latency to stall).
                adde = nc.vector if bi >= len(blocks) - 2 else nc.gpsimd

                npair = (nb + 1) // 2
                prods = [[None] * 5 for _ in range(npair)]
                accs = [[None, None] for _ in range(npair)]
                for m in range(5):
                    pst = [
                        pspool.tile([128, 2 * NTILE], F32, name=f"ps{m}_{p}",
                                    tag="ps")
                        for p in range(npair)
                    ]
                    # chunk 3's window (gtac) is the last DMA to land during
                    # the head, so consume it last
                    for ci, c in enumerate((0, 1, 2, 4, 3)):
                        k = m * 5 + c
                        lhsT = wsb[:, k * 128:(k + 1) * 128]
                        for tb in range(nb):
                            q = tb * NTILE
                            if c < 3:
                                rhs = gtab[:, q + c * W2: q + c * W2 + NTILE]
                            elif c == 3:
                                rhs = gtac[:, q: q + NTILE]
                            else:
                                rhs = gtab[:, q + 262: q + 262 + NTILE]
                            out_ps = pst[tb // 2][:, (tb % 2) * NTILE:
                                                  (tb % 2 + 1) * NTILE]
                            nc.tensor.matmul(
                                out_ps, lhsT, rhs,
                                start=(ci == 0), stop=(ci == 4),
                            )
                    for p in range(npair):
                        q = 2 * p * NTILE
                        Wd = min(2 * NTILE, (nb - 2 * p) * NTILE)
                        pr = prodpool.tile(
                            [128, 2 * NTILE], F16, name=f"m{m}", tag="prod"
                        )
                        prods[p][m] = pr
                        if m < 3:
                            stt(pr[:, 0:Wd], pst[p][:, 0:Wd],
                                bias_sb[:, m:m + 1],
                                grab[:, q + m * W2: q + m * W2 + Wd])
                        elif m == 3:
                            stt(pr[:, 0:Wd], pst[p][:, 0:Wd],
                                bias_sb[:, 3:4],
                                grac[:, q: q + Wd])
                        else:
                            stt(pr[0:64, 0:Wd], pst[p][0:64, 0:Wd],
                                bias_sb[0:64, 4:5],
                                grab[0:64, q + 262: q + 262 + Wd])
                    # eager leaf adds: a1 as soon as m0/m1 products exist,
                    # a2 after m2/m3 — keeps the add engine streaming instead
                    # of bursting at block end
                    if m in (1, 3):
                        for p in range(npair):
                            Wd = min(2 * NTILE, (nb - 2 * p) * NTILE)
                            a = accpool.tile([128, 2 * NTILE], F16,
                                             name=f"a{m}", tag="acc")
                            accs[p][m // 2] = a
                            adde.tensor_tensor(a[:, 0:Wd],
                                               prods[p][m - 1][:, 0:Wd],
                                               prods[p][m][:, 0:Wd], op=ADD)

                for p in range(npair):
                    t = t0 + 2 * p
                    Wd = min(2 * NTILE, (nb - 2 * p) * NTILE)
                    a1, a2 = accs[p]
                    a3 = accpool.tile([128, 2 * NTILE], F16, name="a3",
                                      tag="acc")
                    adde.tensor_tensor(a3[:, 0:Wd], a1[:, 0:Wd],
                                       a2[:, 0:Wd], op=ADD)
                    adde.tensor_tensor(a3[0:64, 0:Wd], a3[0:64, 0:Wd],
                                       prods[p][4][0:64, 0:Wd], op=ADD)
                    nc.scalar.dma_start(
                        out=o_dst[:, t * NTILE: t * NTILE + Wd],
                        in_=a3[:, 0:Wd],
                    )
    nc.finalize()
    return nc


_NC = None


def _get_nc():
    global _NC
    if _NC is None:
        _NC = _build()
    return _NC


_RUN_KW = {}  # test harness can inject trace=True etc.
_LAST_RESULT = None


def kernel(gr, gt, Wc, bc):
    global _LAST_RESULT
    gr = np.ascontiguousarray(np.asarray(gr, dtype=np.float32))
    gt = np.ascontiguousarray(np.asarray(gt, dtype=np.float32))
    Wc = np.asarray(Wc, dtype=np.float32)
    bc = np.asarray(bc, dtype=np.float32)

    wb = _prep_w(Wc)
    bb = _prep_b(bc)
    in_maps = [
        {
            "gt_src": _prep_gt(gt[n]),
            "gr_src": _prep_gr(gr[n]),
            "w_src": wb,
            "b_src": bb,
        }
        for n in range(N)
    ]
    res = run_bass_kernel_spmd(
        _get_nc(), in_maps, core_ids=list(range(N)), **_RUN_KW
    )
    _LAST_RESULT = res

    hh = np.arange(H)
    cols = (hh * W2)[:, None] + np.arange(W)[None, :]
    outs = []
    for n in range(N):
        O = res.results[n]["o_dst"].astype(np.float32)
        flat = O[:64] + O[64:]
        outs.append(flat[:, cols])
    return np.stack(outs).astype(np.float32)


# revision 23
# speedup vs baseline: 1.0244x; 1.0115x over previous
"""Trainium2 bass kernel for nn_CM_41162966565199 (dense_cnn, dynamic filter).

Computation (per batch sample):
  filt = Conv2d(C=64 -> 9C=576, 3x3, pad=1)(gt) + bias          # dynamic filters
  out[c,h,w] = sum_j filt[c*9+j, h, w] * patches_j(gr)[c, h, w] # 3x3 dyn. filter

Strategy: pure data parallel, one sample per NeuronCore (N=8, 8 cores).

Per core:
- Conv as shift-based matmuls in fp16 (full PE rate, half the DMA/SBUF of
  fp32r): contraction (in_channel i, tap p) tiled into 5 K=128 chunks by
  pairing taps whose flat-offset delta is +1 (or +130), realized by stacking
  two shifted copies of gt on SBUF partitions 0-63 / 64-127. Output channels
  (c, j) tiled into 5 M-tiles of two j-groups each. All matmuls K=128, M=128,
  N=512.
- Dynamic-filter stage on DVE: scalar_tensor_tensor fuses (psum + bias) * gr
  reading PSUM directly; the fp16 pairwise product sum tree runs on the
  otherwise-idle GpSimd (Pool) engine so DVE never backs up PSUM recycling.
- Spatial flattening uses a 1-ring padded 130x130 grid so every 3x3 tap is a
  pure flat offset; host pre-pads (zero ring for conv input, the replicate
  ring of gr IS the 130-grid) and crops the 130x130 output grid to 128x128.
  The upper/lower partition halves hold disjoint partial sums, folded on the
  host.
- First block has 1 N-tile so the PE starts after ~0.5 MB of DMA instead of
  ~5 MB; gr/bias DMAs issue from the ACT queue and out-DMAs from the Pool
  queue so the SP queue only carries gt windows + weights.
"""

import numpy as np

import concourse.bass as bass
import concourse.mybir as mybir
import concourse.tile as tile
from concourse import bacc
from concourse.bass_utils import run_bass_kernel_spmd
from concourse.vector_clock import ScopedClock

# ---------------------------------------------------------------- constants
N, C, H, W, KS = 8, 64, 128, 128, 3
W2 = W + 2                      # 130: 1-ring padded row width
NTILE = 512
NT = 33                         # spatial tiles; out o = h*130+w, max 16637
OUT_LEN = NT * NTILE            # 16896
FLAT_SRC = 17160                # padded flat source length (covers max reads)

F32 = mybir.dt.float32
F16 = mybir.dt.float16
ADD = mybir.AluOpType.add
MULT = mybir.AluOpType.mult

# 5 K-chunks over the 9 conv taps p=(kh,kw); flat offset d_p = kh*130+kw.
# Pairs (p_a, p_b): upper/lower SBUF partition halves. Chunks 0-2 pair
# (kh,0)+(kh,1) (delta=1, gtAB buffer), chunk 3 pairs (0,2)+(1,2)
# (delta=130, gtAC buffer), chunk 4 is the lone (2,2) with zeroed lower
# weights.
CHUNKS = [((0, 0), (0, 1)), ((1, 0), (1, 1)), ((2, 0), (2, 1)),
          ((0, 2), (1, 2)), ((2, 2), None)]
# 5 M-tiles: which two j-groups (of the 9 output filter taps) share a PSUM
# tile's upper/lower 64 partitions.
MTILES = CHUNKS

# block sizes (N-tiles per block; windows + weights reused within a block).
# Small first block -> PE starts early; small last blocks -> short tail.
SIZES = [1, 3, 4, 4, 4, 4, 4, 4, 4, 1]
assert sum(SIZES) == NT


# ------------------------------------------------- TileContext drain patch
# This walrus build rejects >2 sync-wait commands on one CTRL instruction;
# the stock TileContext tail hangs every pending sem wait on a single SP
# Drain. Split them across single-wait SP NOPs (program order on SP still
# places them before the barrier + sem reset).
def _drain_and_barrier_split(self, tick_clock, wait_clock):
    nc = self.nc
    drain_inst = nc.sync.drain()
    wait_clock.add_sem_waits(
        drain_inst.ins, ScopedClock({None: tick_clock.global_clock})
    )
    si = drain_inst.ins.sync_info
    if si is not None and len(si.on_wait) > 1:
        waits = list(si.on_wait)
        drain_inst.ins.sync_info = mybir.SyncInfo(on_wait=[waits[0]], on_update=[])
        for w in waits[1:]:
            nop = nc.sync.nop()
            nop.ins.sync_info = mybir.SyncInfo(on_wait=[w], on_update=[])
    nc.all_engine_barrier()
    assert self.sems is not None
    popped = nc._tile_sem_poison_stack.pop()
    assert popped is self._sem_poison
    # The stock tail ends with a second all_engine_barrier after the
    # semaphore clear. It only protects a following tile scope (none here)
    # and costs ~6us of measured exec time (slow staggered barrier); the
    # runtime already waits for every engine to retire, and the clear is the
    # final instruction on its engine, so re-execution stays safe.
    nc.clear_and_free_semaphores(list(self.sems.allocated().values()))


tile.TileContext._drain_and_barrier = _drain_and_barrier_split


# ------------------------------------------------------------- host prep
def _prep_gt(gt):
    """[C,H,W] -> [C, FLAT_SRC] fp16 flat 130x130 grid, 1-ring zero pad."""
    pad = np.zeros((C, W2, W2), np.float16)
    pad[:, 1:1 + H, 1:1 + W] = gt
    buf = np.zeros((C, FLAT_SRC), np.float16)
    buf[:, :W2 * W2] = pad.reshape(C, -1)
    return buf


def _prep_gr(gr):
    """[C,H,W] -> fp16 flat 130x130 grid = replicate-padded gr."""
    rp = np.pad(gr, ((0, 0), (1, 1), (1, 1)), mode="edge")
    buf = np.zeros((C, FLAT_SRC), np.float16)
    buf[:, :W2 * W2] = rp.reshape(C, -1)
    return buf


def _jidx(j):
    return j[0] * 3 + j[1]


def _prep_w(Wc):
    """[576,64,3,3] -> fp16 [128, 25*128] lhsT blocks [(m,chunk), K, M]."""
    out = np.zeros((5, 5, 128, 128), np.float32)
    cc = np.arange(C)
    for m, (j0, j1) in enumerate(MTILES):
        for c, (pa, pb) in enumerate(CHUNKS):
            for hk, p in ((0, pa), (1, pb)):
                if p is None:
                    continue
                kh, kw = p
                for hm, j in ((0, j0), (1, j1)):
                    if j is None:
                        continue
                    blk = Wc[cc * 9 + _jidx(j), :, kh, kw]  # [c_out, i]
                    out[m, c, 64 * hk:64 * hk + 64, 64 * hm:64 * hm + 64] = blk.T
    # partition-major [128, 25*128] so the device load is plain 2D DMAs
    return np.ascontiguousarray(
        out.reshape(25, 128, 128).transpose(1, 0, 2).reshape(128, 25 * 128)
    ).astype(np.float16)


def _prep_b(bc):
    """[576] -> [128,5] per-M-tile per-partition bias (partition-major)."""
    out = np.zeros((5, 128), np.float32)
    cc = np.arange(C)
    for m, (j0, j1) in enumerate(MTILES):
        for hm, j in ((0, j0), (1, j1)):
            if j is None:
                continue
            out[m, 64 * hm:64 * hm + 64] = bc[cc * 9 + _jidx(j)]
    return np.ascontiguousarray(out.T)


# --------------------------------------------------------- bass program
def _build():
    # Bacc (not plain Bass): its finalize() -> compile() legalizes the
    # multi-wait instructions Tile emits (move_matmul_waits_to_ldweights,
    # generate_event_semaphores) which this walrus build otherwise rejects
    # with "Too many sync wait commands".
    nc = bacc.Bacc(None, target_bir_lowering=False)
    gt_src = nc.dram_tensor("gt_src", [C, FLAT_SRC], F16, kind="ExternalInput")
    gr_src = nc.dram_tensor("gr_src", [C, FLAT_SRC], F16, kind="ExternalInput")
    w_src = nc.dram_tensor("w_src", [128, 25 * 128], F16, kind="ExternalInput")
    b_src = nc.dram_tensor("b_src", [128, 5], F32, kind="ExternalInput")
    o_dst = nc.dram_tensor("o_dst", [128, OUT_LEN], F16, kind="ExternalOutput")

    blocks = []
    t0 = 0
    for nb in SIZES:
        blocks.append((t0, nb))
        t0 += nb

    with tile.TileContext(nc) as tc:
        with (
            tc.tile_pool(name="wpool", bufs=1) as wpool,
            tc.tile_pool(name="winpool", bufs=2) as winpool,
            tc.tile_pool(name="pspool", bufs=4, space="PSUM") as pspool,
            tc.tile_pool(name="prodpool", bufs=12) as prodpool,
            tc.tile_pool(name="accpool", bufs=6) as accpool,
        ):
            wsb = wpool.tile([128, 25 * 128], F16, name="wsb", tag="wsb")
            bias_sb = wpool.tile([128, 5], F32, name="bias_sb", tag="bias")

            # PE warmup: the HAM clock gate keeps the PE at 1.2 GHz until it
            # has been busy ~3.4us. Issue small matmuls on a zeroed tile
            # during the head DMA wait; count sized to end right as the first
            # real matmul's inputs land (a gap would re-throttle the PE).
            warm = wpool.tile([128, 64], F16, name="warm", tag="warm")
            nc.gpsimd.memset(warm[:, :], 0.0)
            pswarm = pspool.tile([128, 2 * NTILE], F32, name="pswarm",
                                 tag="ps")
            for _ in range(100):
                nc.tensor.matmul(pswarm[0:64, 0:64], warm[:, :], warm[:, :],
                                 start=True, stop=True)

            def load_weights_m(m):
                nc.sync.dma_start(
                    out=wsb[:, m * 640:(m + 1) * 640],
                    in_=w_src[:, m * 640:(m + 1) * 640],
                )

            def stt(out_ap, ps_ap, b_ap, gr_ap):
                nc.vector.scalar_tensor_tensor(
                    out_ap, ps_ap, b_ap, gr_ap, op0=ADD, op1=MULT
                )

            def win_load(eng, pool, name, src, base, pair_step, dtype, wneed):
                """Partitions 0-63 <- src[base+q], 64-127 <-
                src[base+pair_step+q], as two 2D DMAs of just the columns
                this block touches. eng picks the issuing queue."""
                t = pool.tile([128, 4 * NTILE + 262], dtype, name=name, tag=name)
                eng.dma_start(out=t[0:64, 0:wneed],
                              in_=src[:, base:base + wneed])
                eng.dma_start(
                    out=t[64:128, 0:wneed],
                    in_=src[:, base + pair_step:base + pair_step + wneed],
                )
                return t

            for bi, (t0, nb) in enumerate(blocks):
                T = t0 * NTILE
                wab = nb * NTILE + 262
                wac = nb * NTILE
                # gt windows + weights on SP queue; gr windows + bias on the
                # ACT queue so the first-matmul deps don't queue behind them.
                if bi == 0:
                    # DMA completion is row-descriptor-bound (~3.4us for a
                    # 64-row window regardless of width), so the head is
                    # ordered for earliest first-matmul: w0 + bias lead the
                    # ACT queue while the gt windows lead the SP queue.
                    nc.scalar.dma_start(out=wsb[:, 0:640], in_=w_src[:, 0:640])
                    nc.scalar.dma_start(out=bias_sb[:, :], in_=b_src[:, :])
                gtab = win_load(nc.sync, winpool, "gtab", gt_src, T, 1, F16, wab)
                gtac = win_load(nc.sync, winpool, "gtac", gt_src, T + 2, 130,
                                F16, wac)
                if bi == 0:
                    # remaining weights split to match the m-loop's
                    # consumption ladder (DMA completion is latency-bound);
                    # w1 on SP behind the gt windows, w2-4 on ACT so SP can
                    # issue block1's windows earlier
                    nc.sync.dma_start(out=wsb[:, 640:1280],
                                      in_=w_src[:, 640:1280])
                    nc.scalar.dma_start(out=wsb[:, 1280:1920],
                                        in_=w_src[:, 1280:1920])
                    nc.scalar.dma_start(out=wsb[:, 1920:3200],
                                        in_=w_src[:, 1920:3200])
                grab = win_load(nc.scalar, winpool, "grab", gr_src, T, 1, F16,
                                wab)
                grac = win_load(nc.scalar, winpool, "grac", gr_src, T + 2, 130,
                                F16, wac)

                # Conv matmuls per M-tile (weights reused across the block's
                # N-tiles; the PE reorder window pulls LDWEIGHTS under the
                # previous matmul). PSUM tiles span TWO banks (two adjacent
                # N-tiles) so the product stage runs 1024-wide DVE ops.
                # Products are written fp16 so the Pool add tree runs 2x.
                # fp16 sum tree engine: Pool (slow but fully overlapped) for
                # mid blocks; DVE for the final block so the kernel tail is
                # short (there is no next block for Pool's latency to stall).
                adde = nc.vector if bi >= len(blocks) - 2 else nc.gpsimd

                npair = (nb + 1) // 2
                prods = [[None] * 5 for _ in range(npair)]
                accs = [[None, None] for _ in range(npair)]
                for m in range(5):
                    pst = [
                        pspool.tile([128, 2 * NTILE], F32, name=f"ps{m}_{p}",
                                    tag="ps")
                        for p in range(npair)
                    ]
                    # chunk 3's window (gtac) is the last DMA to land during
                    # the head, so consume it last
                    for ci, c in enumerate((0, 1, 2, 4, 3)):
                        k = m * 5 + c
                        lhsT = wsb[:, k * 128:(k + 1) * 128]
                        for tb in range(nb):
                            q = tb * NTILE
                            if c < 3:
                                rhs = gtab[:, q + c * W2: q + c * W2 + NTILE]
                            elif c == 3:
                                rhs = gtac[:, q: q + NTILE]
                            else:
                                rhs = gtab[:, q + 262: q + 262 + NTILE]
                            out_ps = pst[tb // 2][:, (tb % 2) * NTILE:
                                                  (tb % 2 + 1) * NTILE]
                            nc.tensor.matmul(
                                out_ps, lhsT, rhs,
                                start=(ci == 0), stop=(ci == 4),
                            )
                    for p in range(npair):
                        q = 2 * p * NTILE
                        Wd = min(2 * NTILE, (nb - 2 * p) * NTILE)
                        pr = prodpool.tile(
                            [128, 2 * NTILE], F16, name=f"m{m}", tag="prod"
                        )
                        prods[p][m] = pr
                        if m < 3:
                            stt(pr[:, 0:Wd], pst[p][:, 0:Wd],
                                bias_sb[:, m:m + 1],
                                grab[:, q + m * W2: q + m * W2 + Wd])
                        elif m == 3:
                            stt(pr[:, 0:Wd], pst[p][:, 0:Wd],
                                bias_sb[:, 3:4],
                                grac[:, q: q + Wd])
                        else:
                            stt(pr[0:64, 0:Wd], pst[p][0:64, 0:Wd],
                                bias_sb[0:64, 4:5],
                                grab[0:64, q + 262: q + 262 + Wd])
                    # eager leaf adds: a1 as soon as m0/m1 products exist,
                    # a2 after m2/m3 — keeps the add engine streaming instead
                    # of bursting at block end
                    if m in (1, 3):
                        for p in range(npair):
                            Wd = min(2 * NTILE, (nb - 2 * p) * NTILE)
                            a = accpool.tile([128, 2 * NTILE], F16,
                                             name=f"a{m}", tag="acc")
                            accs[p][m // 2] = a
                            adde.tensor_tensor(a[:, 0:Wd],
                                               prods[p][m - 1][:, 0:Wd],
                                               prods[p][m][:, 0:Wd], op=ADD)

                for p in range(npair):
                    t = t0 + 2 * p
                    Wd = min(2 * NTILE, (nb - 2 * p) * NTILE)
                    a1, a2 = accs[p]
                    a3 = accpool.tile([128, 2 * NTILE], F16, name="a3",
                                      tag="acc")
                    adde.tensor_tensor(a3[:, 0:Wd], a1[:, 0:Wd],
                                       a2[:, 0:Wd], op=ADD)
                    adde.tensor_tensor(a3[0:64, 0:Wd], a3[0:64, 0:Wd],
                                       prods[p][4][0:64, 0:Wd], op=ADD)
                    nc.scalar.dma_start(
                        out=o_dst[:, t * NTILE: t * NTILE + Wd],
                        in_=a3[:, 0:Wd],
                    )
    nc.finalize()
    return nc


_NC = None


def _get_nc():
    global _NC
    if _NC is None:
        _NC = _build()
    return _NC


_RUN_KW = {}  # test harness can inject trace=True etc.
_LAST_RESULT = None


def kernel(gr, gt, Wc, bc):
    global _LAST_RESULT
    gr = np.ascontiguousarray(np.asarray(gr, dtype=np.float32))
    gt = np.ascontiguousarray(np.asarray(gt, dtype=np.float32))
    Wc = np.asarray(Wc, dtype=np.float32)
    bc = np.asarray(bc, dtype=np.float32)

    wb = _prep_w(Wc)
    bb = _prep_b(bc)
    in_maps = [
        {
            "gt_src": _prep_gt(gt[n]),
            "gr_src": _prep_gr(gr[n]),
            "w_src": wb,
            "b_src": bb,
        }
        for n in range(N)
    ]
    res = run_bass_kernel_spmd(
        _get_nc(), in_maps, core_ids=list(range(N)), **_RUN_KW
    )
    _LAST_RESULT = res

    hh = np.arange(H)
    cols = (hh * W2)[:, None] + np.arange(W)[None, :]
    outs = []
    for n in range(N):
        O = res.results[n]["o_dst"].astype(np.float32)
        flat = O[:64] + O[64:]
        outs.append(flat[:, cols])
    return np.stack(outs).astype(np.float32)
